# revision 11
# baseline (speedup 1.0000x reference)
"""Trainium2 Bass kernel for nn_LowRankConv3D (CP-decomposed 3x3x3 conv).

Math (reference): out[b,co,h,w,d] =
    sum_{c,kh,kw,kd,r} x[b,c,h+kh-1,w+kw-1,d+kd-1]
      * U_c_in[c,r] U_k_h[kh,r] U_k_w[kw,r] U_k_d[kd,r] U_c_out[r,co]  + bias[co]

Kernel decomposition (per core):
  Stage A (PE): t2[r, h,w,d] = sum_{c,kh} W1[(c,kh),r] x[c, h+kh-1, w, d]
     -> 3 accumulating K=32 matmuls per 512-col chunk, one per kh, where the
        kh shift selects a different x h-plane tile. Output M=128 (rank block
        duplicated to both partition halves) so stage B can run on both
        64-row PE tiles.
  Evac (ScalarE): PSUM -> padded SBUF plane buffer [128, 66, 66] (zero halo).
  Stage B (PE): out[co, chunk] = sum_{(kw,kd), r} W2[(kw,kd),r,co]
        * t2[r, w+kw-1, d+kd-1]
     -> 9 accumulating K=64 matmuls per chunk; (kw,kd) shifts are free-dim
        offsets into the padded plane buffer.
  Out-evac (VectorE): PSUM + bias -> SBUF, then DMA to HBM.

Sharding: 8 cores = batch (2) x h-quarter (4). Each core: 16 output h-planes,
x slice of 18 h-planes (halo, zero-padded at the global h edge).
Factor matrices are folded on the host into W1 [3,32,128] / W2 [9,64,64] and
replicated across partition groups.
"""

import sys

sys.path.insert(0, "/opt/trn_rl_repo")

import numpy as np

B, C_IN, C_OUT, RNK, K = 2, 32, 64, 64, 3
H = W = D = 64
HQ = 16          # output h-planes per core
NPLANES = HQ + 2  # x planes incl. halo
NCH = 8          # chunks per plane
NFD = 512        # free size per chunk (8 w-rows x 64 d)
WP = 66          # padded plane dims

MM_DT = "bfloat16"   # matmul streaming dtype (1 col/cycle, ldweights path)

_cached = {}


def _build_bass():
    import concourse.bass as bass
    import concourse.mybir as mybir
    import concourse.tile as tile

    f32 = mybir.dt.float32
    mmdt = getattr(mybir.dt, MM_DT)

    nc = bass.Bass(target_bir_lowering=False)
    x_h = nc.declare_dram_parameter("x", [NPLANES, 128, 1024], mmdt, isOutput=False)
    w1_h = nc.declare_dram_parameter("w1", [128, K, 2, C_OUT], mmdt, isOutput=False)
    w2_h = nc.declare_dram_parameter("w2", [128, 9, C_OUT], mmdt, isOutput=False)
    b_h = nc.declare_dram_parameter("bias", [128, 1], f32, isOutput=False)
    out_h = nc.declare_dram_parameter(
        "out", [HQ, NCH, C_OUT, NFD], f32, isOutput=True
    )

    with tile.TileContext(nc) as tc:
        with (
            tc.tile_pool(name="xp", bufs=1) as xp,
            tc.tile_pool(name="wp", bufs=1) as wp,
            tc.tile_pool(name="t2pl", bufs=1) as t2plp,
            tc.tile_pool(name="osb", bufs=6) as osbp,
            tc.tile_pool(name="t2ps", bufs=4, space="PSUM") as t2psp,
            tc.tile_pool(name="ops", bufs=4, space="PSUM") as opsp,
        ):
            # ---- constants ----
            # w1p[(half*64)+r, kh, sel, m]: K=64 zero-padded stage-A weights.
            # sel=0: rows 0-31 hold W1 (x quarter at the low half of the row
            # tile), sel=1: rows 32-63 (x quarter at the high half).
            w1_sb = wp.tile([128, K, 2, C_OUT], mmdt, tag="w1")
            w2_sb = wp.tile([128, 9, C_OUT], mmdt, tag="w2")
            bias_sb = wp.tile([128, 1], f32, tag="bias")
            nc.sync.dma_start(out=w1_sb, in_=w1_h[:])
            nc.sync.dma_start(out=w2_sb, in_=w2_h[:])
            nc.sync.dma_start(out=bias_sb, in_=b_h[:])

            # ---- x planes ----
            x_tiles = []
            for hp in range(NPLANES):
                xt = xp.tile([128, 1024], mmdt, tag=f"x{hp}")
                nc.sync.dma_start(out=xt, in_=x_h[hp])
                x_tiles.append(xt)

            # ---- t2 plane ring buffers (padded, zero halo) ----
            t2pl = []
            for i in range(3):
                t = t2plp.tile([128, WP, WP], mmdt, tag=f"t2pl{i}")
                nc.gpsimd.memset(t, 0.0)
                t2pl.append(t)

            taps = [(kw, kd) for kw in range(K) for kd in range(K)]

            # All matmuls are tile_size (64, 64): uniform PE tiling mode (no
            # mode-switch drains), and every accumulation group stays on ONE
            # row tile (two row tiles must never target the same PSUM
            # bank+partition range concurrently).
            for h in range(HQ):
                pl = t2pl[h % 3]
                t2ps_c = []
                # ---- stage A: channel+h-tap contraction ----
                for c in range(NCH):
                    q = c // 2
                    base, sel = 64 * (q // 2), q % 2
                    fd0 = (c % 2) * NFD
                    ps = t2psp.tile([128, NCH, D], f32)
                    for ch in (0, 64):
                        for kh in range(K):
                            nc.tensor.matmul(
                                out=ps[ch : ch + C_OUT, :, :],
                                lhsT=w1_sb[base : base + 64, kh, sel, :],
                                rhs=x_tiles[h + kh][
                                    base : base + 64, fd0 : fd0 + NFD
                                ],
                                start=(kh == 0),
                                stop=(kh == K - 1),
                                tile_position=(base, ch),
                            )
                    t2ps_c.append(ps)
                # ---- evac to padded plane (ScalarE) ----
                for c in range(NCH):
                    nc.scalar.copy(
                        out=pl[:, 1 + 8 * c : 9 + 8 * c, 1 : 1 + D],
                        in_=t2ps_c[c][:, :, :],
                    )
                # ---- stage B: 9 fused (w,d)-tap x expand matmuls ----
                for c in range(NCH):
                    rh = 64 * (c % 2)
                    ch = 64 * ((c // 2) % 2)
                    ops = opsp.tile([128, NFD], f32)
                    for i, (kw, kd) in enumerate(taps):
                        nc.tensor.matmul(
                            out=ops[ch : ch + C_OUT, :],
                            lhsT=w2_sb[rh : rh + RNK, i, :],
                            rhs=pl[
                                rh : rh + RNK, 8 * c + kw : 8 * c + kw + 8, kd : kd + D
                            ],
                            start=(i == 0),
                            stop=(i == len(taps) - 1),
                            tile_position=(rh, ch),
                        )
                    # ---- bias add + evac (VectorE), then DMA out ----
                    osb = osbp.tile([128, NFD], f32)
                    nc.vector.tensor_scalar_add(
                        out=osb[ch : ch + C_OUT, :],
                        in0=ops[ch : ch + C_OUT, :],
                        scalar1=bias_sb[ch : ch + C_OUT, :],
                    )
                    nc.sync.dma_start(
                        out=out_h[h, c], in_=osb[ch : ch + C_OUT, :]
                    )
    _split_waits(nc)
    return nc


def _split_waits(nc):
    """Walrus allows only one sync-wait command on compute instructions in
    this flow and nothing downstream splits them, so hoist extra waits onto
    same-engine NoOps (engine blocks on each sequentially)."""
    import concourse.mybir as mybir

    n = 0
    for fn in nc.m.functions:
        for blk in fn.blocks:
            out = []
            for inst in blk.instructions:
                si = inst.sync_info
                if si is not None and len(si.on_wait) > 1:
                    waits = list(si.on_wait)
                    for w in waits[:-1]:
                        nop = mybir.InstNoOp(
                            name=f"I-waitsplit-{n}",
                            sync_info=mybir.SyncInfo(on_wait=[w], on_update=[]),
                            engine=inst.engine,
                            bass_nofuse=True,
                        )
                        n += 1
                        out.append(nop)
                    si.on_wait = [waits[-1]]
                out.append(inst)
            blk.instructions[:] = out


def _prep_inputs(x, U_k_h, U_k_w, U_k_d, U_c_in, U_c_out, bias):
    x = np.asarray(x, dtype=np.float32)
    xp = np.pad(x, ((0, 0), (0, 0), (1, 1), (0, 0), (0, 0)))

    w1 = np.einsum("cr,kr->kcr", np.asarray(U_c_in, np.float32),
                   np.asarray(U_k_h, np.float32))          # [3,32,64]
    w1p = np.zeros((64, 3, 2, 64), np.float32)
    w1p[:32, :, 0, :] = w1.transpose(1, 0, 2)               # sel=0: low rows
    w1p[32:, :, 1, :] = w1.transpose(1, 0, 2)               # sel=1: high rows
    w1_full = np.ascontiguousarray(np.tile(w1p, (2, 1, 1, 1)))  # [128,3,2,64]
    w2 = np.einsum("kr,lr,rc->klrc", np.asarray(U_k_w, np.float32),
                   np.asarray(U_k_d, np.float32),
                   np.asarray(U_c_out, np.float32)).reshape(9, RNK, C_OUT)
    w2_full = np.ascontiguousarray(np.tile(w2.transpose(1, 0, 2), (2, 1, 1)))
    bias_full = np.ascontiguousarray(
        np.tile(np.asarray(bias, np.float32)[:, None], (2, 1))
    )

    import ml_dtypes
    bf16 = ml_dtypes.bfloat16
    w1_full = w1_full.astype(bf16)
    w2_full = w2_full.astype(bf16)
    in_maps = []
    for core in range(8):
        b, q = divmod(core, 4)
        xs = xp[b, :, 16 * q : 16 * q + NPLANES]            # [32,18,64,64]
        xs = xs.reshape(C_IN, NPLANES, 4, 16, D)            # c,hp,wq,w',d
        xs = np.ascontiguousarray(
            xs.transpose(1, 2, 0, 3, 4).reshape(NPLANES, 128, 1024)
        ).astype(bf16)
        in_maps.append({"x": xs, "w1": w1_full, "w2": w2_full, "bias": bias_full})
    return in_maps


def _assemble(results):
    y = np.empty((B, C_OUT, H, W, D), dtype=np.float32)
    for core in range(8):
        b, q = divmod(core, 4)
        o = np.asarray(results[core]["out"], np.float32)    # [16,8,64,512]
        o = o.reshape(HQ, NCH, C_OUT, 8, D).transpose(2, 0, 1, 3, 4)
        y[b, :, 16 * q : 16 * q + HQ] = o.reshape(C_OUT, HQ, W, D)
    return y


def kernel(x, U_k_h, U_k_w, U_k_d, U_c_in, U_c_out, bias, _trace=False):
    from concourse.bass_utils import run_bass_kernel_spmd

    if "nc" not in _cached:
        _cached["nc"] = _build_bass()
    in_maps = _prep_inputs(x, U_k_h, U_k_w, U_k_d, U_c_in, U_c_out, bias)
    res = run_bass_kernel_spmd(
        _cached["nc"], in_maps, list(range(8)), trace=_trace
    )
    _cached["last_result"] = res
    return _assemble(res.results)
